# revision 8
# baseline (speedup 1.0000x reference)
"""GraphSAGE 2-layer kernel for Trainium2, 8 NeuronCores, data-parallel over nodes.

Strategy (v2 — dma_gather):
- Nodes padded to 50176 = 8 cores * 49 tiles * 128; each core owns 6272 rows.
- One compiled Bass program = one SAGE layer for one core's slice, computing
    out[:, n] = relu(W_top^T x_n + W_bot_scaled^T (sum_k x[nbr[n,k]]) + b)
  i.e. the OUTPUT IS TRANSPOSED [128 outf, npc nodes]; host stitches/casts
  between the two layer launches (host work is not on the HW critical path).
- Neighbor gather via the mlp-library dma_gather (InstDMAGatherAnt):
  * table packed in pairs: t2[q] = concat(x[2q], x[2q+1]) (512B f16 rows), so
    the int16 gather index q = node//2 stays < 32768; a parity mask selects
    the wanted 256B half on DVE after the gather.
  * two 1024-index gathers per 128-node tile (16 neighbors each); 1024 is the
    max num_idxs that executes reliably per instruction.
- Sum over 16 neighbors: parity select (copy + copy_predicated) then a 4-level
  tree of wide DVE adds; mean folded into pre-scaled W_bot.
- s^T via PE identity-transpose (f32), then two f16 matmuls accumulate
  W_top^T x + W_bot^T s in PSUM; bias+relu fused on ACT (bias is per-partition
  in the transposed orientation).
"""

import sys

sys.path.insert(0, "/opt/trn_rl_repo")

from contextlib import ExitStack

import numpy as np

import concourse.bass as bass
import concourse.tile as tile
from concourse import library_config, mybir
from concourse.bass_utils import run_bass_kernel_spmd
from concourse.library_overlay import lower_extended_insts
from concourse.masks import make_identity

P = 128
DEG = 16
C = 8
GN = 1024  # indices per dma_gather instruction
PADF = 8  # free-dim pad (f16 elems) after each 128-wide chunk

f32 = mybir.dt.float32
f16 = mybir.dt.float16
i16 = mybir.dt.int16
u8 = mybir.dt.uint8

_NC_CACHE = {}


def _split_wide_waits(nc, max_waits=1):
    """walrus codegen here allows a single sync-wait per instruction; move
    extra waits onto preceding nops on the same engine queue."""
    for fn in nc.m.functions:
        for bb in fn.blocks:
            out = []
            for inst in bb.instructions:
                si = inst.sync_info
                ow = list(si.on_wait) if si and si.on_wait else []
                limit = 0 if isinstance(inst, mybir.InstDrain) else max_waits
                if len(ow) > limit:
                    extra = ow if limit == 0 else ow[:-limit]
                    keep = [] if limit == 0 else ow[-limit:]
                    for k in range(0, len(extra), max_waits):
                        out.append(
                            mybir.InstNoOp(
                                name=f"{inst.name}-waitsplit{k}",
                                opcode="Nop",
                                engine=inst.engine,
                                debug=inst.debug,
                                ins=[],
                                outs=[],
                                sync_info=mybir.SyncInfo(
                                    on_wait=extra[k : k + max_waits], on_update=[]
                                ),
                                text_hint="waitsplit",
                                bass_nofuse=True,
                            )
                        )
                    si.on_wait = keep
                out.append(inst)
            bb.instructions[:] = out


def build_layer_nc(n_tiles, n_full):
    key = (n_tiles, n_full)
    if key in _NC_CACHE:
        return _NC_CACHE[key]
    npc = n_tiles * P
    nc = bass.Bass(
        "TRN2",
        target_bir_lowering=False,
        debug=False,
        num_devices=C,
        num_swdge_queues=4,
    )
    t2 = nc.dram_tensor("t2", [n_full // 2, 2 * P], f16, kind="ExternalInput").ap()
    xt = nc.dram_tensor("xt", [P, npc], f16, kind="ExternalInput").ap()
    idx = nc.dram_tensor("idx", [P, n_tiles * P], i16, kind="ExternalInput").ap()
    par = nc.dram_tensor("par", [P, n_tiles * DEG], u8, kind="ExternalInput").ap()
    wt = nc.dram_tensor("wt", [P, P], f16, kind="ExternalInput").ap()
    wb = nc.dram_tensor("wb", [P, P], f16, kind="ExternalInput").ap()
    bia = nc.dram_tensor("bia", [P, 1], f32, kind="ExternalInput").ap()
    out = nc.dram_tensor("out", [P, npc], f32, kind="ExternalOutput").ap()

    with tile.TileContext(nc) as tc:
        with ExitStack() as ctx:
            consts = ctx.enter_context(tc.tile_pool(name="consts", bufs=1))
            gpool = ctx.enter_context(tc.tile_pool(name="gath", bufs=4))
            t8p = ctx.enter_context(tc.tile_pool(name="t8p", bufs=2))
            t4p = ctx.enter_context(tc.tile_pool(name="t4p", bufs=2))
            t2p = ctx.enter_context(tc.tile_pool(name="t2p", bufs=2))
            t1p = ctx.enter_context(tc.tile_pool(name="t1p", bufs=2))
            stp = ctx.enter_context(tc.tile_pool(name="stp", bufs=2))
            hpool = ctx.enter_context(tc.tile_pool(name="hh", bufs=3))
            ps_t = ctx.enter_context(tc.tile_pool(name="ps_t", bufs=2, space="PSUM"))
            ps_h = ctx.enter_context(tc.tile_pool(name="ps_h", bufs=3, space="PSUM"))

            nc.gpsimd.load_library(library_config.mlp)
            gn_reg = nc.gpsimd.to_reg(GN)

            ident = consts.tile([P, P], f32)
            make_identity(nc, ident[:])
            wt_sb = consts.tile([P, P], f16)
            nc.sync.dma_start(wt_sb[:], wt[:, :])
            wb_sb = consts.tile([P, P], f16)
            nc.sync.dma_start(wb_sb[:], wb[:, :])
            b_sb = consts.tile([P, 1], f32)
            nc.sync.dma_start(b_sb[:], bia[:, :])
            xt_sb = consts.tile([P, npc], f16)
            nc.sync.dma_start(xt_sb[:], xt[:, :])
            idx_sb = consts.tile([P, n_tiles * P], i16)
            nc.sync.dma_start(idx_sb[:], idx[:, :])
            par_sb = consts.tile([P, n_tiles * DEG], u8)
            nc.sync.dma_start(par_sb[:], par[:, :])

            for t in range(n_tiles):
                # two half-tile gathers into separate tiles so their DMA
                # transfers overlap (no WAW sem serialization)
                gs = []
                for h in range(2):
                    g = gpool.tile([P, 8, 2 * P], f16)
                    nc.gpsimd.dma_gather(
                        out_ap=g[:],
                        in_ap=t2[:, :],
                        idxs_ap=idx_sb[:, t * P + 64 * h : t * P + 64 * (h + 1)],
                        num_idxs=GN,
                        num_idxs_reg=gn_reg,
                        elem_size=2 * P,
                        queue_num=(2 * t + h) % 4,
                    )
                    gs.append(g)
                # parity select in place: odd half onto even half where par=1
                for h in range(2):
                    nc.vector.copy_predicated(
                        gs[h][:, :, 0:P],
                        par_sb[:, t * DEG + 8 * h : t * DEG + 8 * h + 8]
                        .unsqueeze(2)
                        .broadcast_to([P, 8, P]),
                        gs[h][:, :, P : 2 * P],
                    )
                # tree sum over 16 chunks (PADF keeps ranks consistent for sim)
                s8 = t8p.tile([P, 8, P + PADF], f16)
                nc.vector.tensor_add(
                    s8[:, :, 0:P], gs[0][:, :, 0:P], gs[1][:, :, 0:P]
                )
                s4 = t4p.tile([P, 4, P + PADF], f16)
                nc.vector.tensor_add(
                    s4[:, :, 0:P], s8[:, 0:4, 0:P], s8[:, 4:8, 0:P]
                )
                s2 = t2p.tile([P, 2, P + PADF], f16)
                nc.vector.tensor_add(
                    s2[:, :, 0:P], s4[:, 0:2, 0:P], s4[:, 2:4, 0:P]
                )
                s1 = t1p.tile([P, P], f32)
                nc.vector.tensor_add(s1[:], s2[:, 0, 0:P], s2[:, 1, 0:P])

                pst = ps_t.tile([P, P], f32)
                nc.tensor.transpose(pst[:], s1[:], ident[:])
                sT = stp.tile([P, P], f16)
                nc.scalar.copy(sT[:], pst[:])

                psh = ps_h.tile([P, P], f32)
                nc.tensor.matmul(
                    out=psh[:],
                    lhsT=wt_sb[:],
                    rhs=xt_sb[:, t * P : (t + 1) * P],
                    start=True,
                    stop=False,
                )
                nc.tensor.matmul(
                    out=psh[:], lhsT=wb_sb[:], rhs=sT[:], start=False, stop=True
                )
                hb = hpool.tile([P, P], f32)
                nc.scalar.activation(
                    hb[:], psh[:], mybir.ActivationFunctionType.Relu, bias=b_sb[:]
                )
                nc.sync.dma_start(out[:, t * P : (t + 1) * P], hb[:])

    _split_wide_waits(nc)
    lower_extended_insts(nc)
    _NC_CACHE[key] = nc
    return nc


def _make_idx_par(nbr_pad, npc, n_tiles):
    """Per-core gather index + parity layouts.

    nbr_pad: [n_full, DEG] int64/int32 neighbor table (full).
    Returns per-core lists: idx [128, n_tiles*128] i16, par [128, n_tiles*16] u8.
    Slot s (within a 1024-idx gather h of tile t) = k_local*128 + n, with
    k = 8*h + k_local; idx value = nbr//2 placed at
    [s%16 (replicated across the 8 16-partition groups), t*128 + 64*h + s//16].
    """
    idxs, pars = [], []
    for c in range(C):
        nb = nbr_pad[c * npc : (c + 1) * npc].reshape(n_tiles, P, DEG)
        a = (nb // 2).astype(np.int16).transpose(0, 2, 1)  # [t, k, n]
        a = a.reshape(n_tiles, 2, 8, 8, 16)  # [t, h, k_l, n_hi, n_lo]
        b = a.transpose(4, 0, 1, 2, 3).reshape(16, n_tiles * P)
        idxs.append(np.ascontiguousarray(np.tile(b, (8, 1))))
        p = (nb & 1).astype(np.uint8).transpose(1, 0, 2).reshape(P, n_tiles * DEG)
        pars.append(np.ascontiguousarray(p))
    return idxs, pars


def _run_layer(nc, table_f16, xt_slices, idxs, pars, W, b, trace=False):
    wt = np.ascontiguousarray(W[:P, :]).astype(np.float16)
    wb = (np.ascontiguousarray(W[P:, :]) / np.float32(DEG)).astype(np.float16)
    bia = np.asarray(b, dtype=np.float32).reshape(P, 1)
    t2 = table_f16.reshape(table_f16.shape[0] // 2, 2 * P)
    in_maps = []
    for c in range(C):
        in_maps.append(
            {
                "t2": t2,
                "xt": xt_slices[c],
                "idx": idxs[c],
                "par": pars[c],
                "wt": wt,
                "wb": wb,
                "bia": bia,
            }
        )
    res = run_bass_kernel_spmd(nc, in_maps, core_ids=list(range(C)), trace=trace)
    # h^T stitched: [128, n_full] f32
    ht = np.concatenate([res.results[c]["out"] for c in range(C)], axis=1)
    return ht, res


LAST_RUNS = []


def kernel(x, neighbors, W1, b1, W2, b2):
    N, D = x.shape
    assert D == P
    npc = -(-N // (C * P)) * P  # rows per core, padded to 128
    n_full = C * npc
    n_tiles = npc // P

    xp = np.zeros((n_full, P), dtype=np.float32)
    xp[:N] = np.asarray(x, dtype=np.float32)
    nbr_pad = np.zeros((n_full, DEG), dtype=np.int64)
    nbr_pad[:N] = np.asarray(neighbors)

    idxs, pars = _make_idx_par(nbr_pad, npc, n_tiles)
    nc = build_layer_nc(n_tiles, n_full)

    x16 = xp.astype(np.float16)
    xt1 = [
        np.ascontiguousarray(x16[c * npc : (c + 1) * npc].T) for c in range(C)
    ]
    h1t, r1 = _run_layer(nc, x16, xt1, idxs, pars, W1, b1)

    h1_16 = h1t.astype(np.float16)  # [128, n_full]
    table2 = np.ascontiguousarray(h1_16.T)  # [n_full, 128] node-major
    xt2 = [
        np.ascontiguousarray(h1_16[:, c * npc : (c + 1) * npc]) for c in range(C)
    ]
    h2t, r2 = _run_layer(nc, table2, xt2, idxs, pars, W2, b2)

    LAST_RUNS[:] = [r1, r2]
    return np.ascontiguousarray(h2t.T[:N]).astype(np.float32)


# revision 9
# speedup vs baseline: 1.8199x; 1.8199x over previous
"""GraphSAGE 2-layer kernel for Trainium2, 8 NeuronCores, data-parallel over nodes.

Strategy (v2 — dma_gather):
- Nodes padded to 50176 = 8 cores * 49 tiles * 128; each core owns 6272 rows.
- One compiled Bass program = one SAGE layer for one core's slice, computing
    out[:, n] = relu(W_top^T x_n + W_bot_scaled^T (sum_k x[nbr[n,k]]) + b)
  i.e. the OUTPUT IS TRANSPOSED [128 outf, npc nodes]; host stitches/casts
  between the two layer launches (host work is not on the HW critical path).
- Neighbor gather via the mlp-library dma_gather (InstDMAGatherAnt):
  * table packed in pairs: t2[q] = concat(x[2q], x[2q+1]) (512B f16 rows), so
    the int16 gather index q = node//2 stays < 32768; a parity mask selects
    the wanted 256B half on DVE after the gather.
  * two 1024-index gathers per 128-node tile (16 neighbors each); 1024 is the
    max num_idxs that executes reliably per instruction.
- Sum over 16 neighbors: parity select (copy + copy_predicated) then a 4-level
  tree of wide DVE adds; mean folded into pre-scaled W_bot.
- s^T via PE identity-transpose (f32), then two f16 matmuls accumulate
  W_top^T x + W_bot^T s in PSUM; bias+relu fused on ACT (bias is per-partition
  in the transposed orientation).
"""

import sys

sys.path.insert(0, "/opt/trn_rl_repo")

from contextlib import ExitStack

import numpy as np

import concourse.bass as bass
import concourse.tile as tile
from concourse import library_config, mybir
from concourse.bass_utils import run_bass_kernel_spmd
from concourse.library_overlay import lower_extended_insts
from concourse.masks import make_identity

P = 128
DEG = 16
C = 8
GN = 1024  # indices per dma_gather instruction
PADF = 8  # free-dim pad (f16 elems) after each 128-wide chunk

f32 = mybir.dt.float32
f16 = mybir.dt.float16
i16 = mybir.dt.int16
u8 = mybir.dt.uint8

_NC_CACHE = {}


def _split_wide_waits(nc, max_waits=1):
    """walrus codegen here allows a single sync-wait per instruction; move
    extra waits onto preceding nops on the same engine queue."""
    for fn in nc.m.functions:
        for bb in fn.blocks:
            out = []
            for inst in bb.instructions:
                si = inst.sync_info
                ow = list(si.on_wait) if si and si.on_wait else []
                limit = 0 if isinstance(inst, mybir.InstDrain) else max_waits
                if len(ow) > limit:
                    extra = ow if limit == 0 else ow[:-limit]
                    keep = [] if limit == 0 else ow[-limit:]
                    for k in range(0, len(extra), max_waits):
                        out.append(
                            mybir.InstNoOp(
                                name=f"{inst.name}-waitsplit{k}",
                                opcode="Nop",
                                engine=inst.engine,
                                debug=inst.debug,
                                ins=[],
                                outs=[],
                                sync_info=mybir.SyncInfo(
                                    on_wait=extra[k : k + max_waits], on_update=[]
                                ),
                                text_hint="waitsplit",
                                bass_nofuse=True,
                            )
                        )
                    si.on_wait = keep
                out.append(inst)
            bb.instructions[:] = out


def build_layer_nc(n_tiles, n_full):
    key = (n_tiles, n_full)
    if key in _NC_CACHE:
        return _NC_CACHE[key]
    npc = n_tiles * P
    nc = bass.Bass(
        "TRN2",
        target_bir_lowering=False,
        debug=False,
        num_devices=C,
        num_swdge_queues=4,
    )
    t2 = nc.dram_tensor("t2", [n_full // 2, 2 * P], f16, kind="ExternalInput").ap()
    xt = nc.dram_tensor("xt", [P, npc], f16, kind="ExternalInput").ap()
    idx = nc.dram_tensor("idx", [P, n_tiles * P], i16, kind="ExternalInput").ap()
    par = nc.dram_tensor("par", [P, n_tiles * DEG], u8, kind="ExternalInput").ap()
    wt = nc.dram_tensor("wt", [P, P], f16, kind="ExternalInput").ap()
    wb = nc.dram_tensor("wb", [P, P], f16, kind="ExternalInput").ap()
    bia = nc.dram_tensor("bia", [P, 1], f32, kind="ExternalInput").ap()
    out = nc.dram_tensor("out", [P, npc], f32, kind="ExternalOutput").ap()

    with tile.TileContext(nc) as tc:
        with ExitStack() as ctx:
            consts = ctx.enter_context(tc.tile_pool(name="consts", bufs=1))
            gpool = ctx.enter_context(tc.tile_pool(name="gath", bufs=12))
            t8p = ctx.enter_context(tc.tile_pool(name="t8p", bufs=3))
            t4p = ctx.enter_context(tc.tile_pool(name="t4p", bufs=3))
            t2p = ctx.enter_context(tc.tile_pool(name="t2p", bufs=3))
            t1p = ctx.enter_context(tc.tile_pool(name="t1p", bufs=3))
            stp = ctx.enter_context(tc.tile_pool(name="stp", bufs=2))
            hpool = ctx.enter_context(tc.tile_pool(name="hh", bufs=3))
            ps_t = ctx.enter_context(tc.tile_pool(name="ps_t", bufs=2, space="PSUM"))
            ps_h = ctx.enter_context(tc.tile_pool(name="ps_h", bufs=3, space="PSUM"))

            nc.gpsimd.load_library(library_config.mlp)
            gn_reg = nc.gpsimd.to_reg(GN)

            ident = consts.tile([P, P], f32)
            make_identity(nc, ident[:])
            wt_sb = consts.tile([P, P], f16)
            nc.sync.dma_start(wt_sb[:], wt[:, :])
            wb_sb = consts.tile([P, P], f16)
            nc.sync.dma_start(wb_sb[:], wb[:, :])
            b_sb = consts.tile([P, 1], f32)
            nc.sync.dma_start(b_sb[:], bia[:, :])
            xt_sb = consts.tile([P, npc], f16)
            nc.sync.dma_start(xt_sb[:], xt[:, :])
            idx_sb = consts.tile([P, n_tiles * P], i16)
            nc.sync.dma_start(idx_sb[:], idx[:, :])
            par_sb = consts.tile([P, n_tiles * DEG], u8)
            nc.sync.dma_start(par_sb[:], par[:, :])

            for t in range(n_tiles):
                # two half-tile gathers into separate tiles so their DMA
                # transfers overlap (no WAW sem serialization)
                gs = []
                for h in range(2):
                    g = gpool.tile([P, 8, 2 * P], f16)
                    nc.gpsimd.dma_gather(
                        out_ap=g[:],
                        in_ap=t2[:, :],
                        idxs_ap=idx_sb[:, t * P + 64 * h : t * P + 64 * (h + 1)],
                        num_idxs=GN,
                        num_idxs_reg=gn_reg,
                        elem_size=2 * P,
                        queue_num=(2 * t + h) % 4,
                    )
                    gs.append(g)
                # parity select in place: odd half onto even half where par=1
                for h in range(2):
                    nc.vector.copy_predicated(
                        gs[h][:, :, 0:P],
                        par_sb[:, t * DEG + 8 * h : t * DEG + 8 * h + 8]
                        .unsqueeze(2)
                        .broadcast_to([P, 8, P]),
                        gs[h][:, :, P : 2 * P],
                    )
                # tree sum over 16 chunks (PADF keeps ranks consistent for sim)
                s8 = t8p.tile([P, 8, P + PADF], f16)
                nc.vector.tensor_add(
                    s8[:, :, 0:P], gs[0][:, :, 0:P], gs[1][:, :, 0:P]
                )
                s4 = t4p.tile([P, 4, P + PADF], f16)
                nc.vector.tensor_add(
                    s4[:, :, 0:P], s8[:, 0:4, 0:P], s8[:, 4:8, 0:P]
                )
                s2 = t2p.tile([P, 2, P + PADF], f16)
                nc.vector.tensor_add(
                    s2[:, :, 0:P], s4[:, 0:2, 0:P], s4[:, 2:4, 0:P]
                )
                s1 = t1p.tile([P, P], f32)
                nc.vector.tensor_add(s1[:], s2[:, 0, 0:P], s2[:, 1, 0:P])

                pst = ps_t.tile([P, P], f32)
                nc.tensor.transpose(pst[:], s1[:], ident[:])
                sT = stp.tile([P, P], f16)
                nc.scalar.copy(sT[:], pst[:])

                psh = ps_h.tile([P, P], f32)
                nc.tensor.matmul(
                    out=psh[:],
                    lhsT=wt_sb[:],
                    rhs=xt_sb[:, t * P : (t + 1) * P],
                    start=True,
                    stop=False,
                )
                nc.tensor.matmul(
                    out=psh[:], lhsT=wb_sb[:], rhs=sT[:], start=False, stop=True
                )
                hb = hpool.tile([P, P], f32)
                nc.scalar.activation(
                    hb[:], psh[:], mybir.ActivationFunctionType.Relu, bias=b_sb[:]
                )
                nc.sync.dma_start(out[:, t * P : (t + 1) * P], hb[:])

    _split_wide_waits(nc)
    lower_extended_insts(nc)
    _NC_CACHE[key] = nc
    return nc


def _make_idx_par(nbr_pad, npc, n_tiles):
    """Per-core gather index + parity layouts.

    nbr_pad: [n_full, DEG] int64/int32 neighbor table (full).
    Returns per-core lists: idx [128, n_tiles*128] i16, par [128, n_tiles*16] u8.
    Slot s (within a 1024-idx gather h of tile t) = k_local*128 + n, with
    k = 8*h + k_local; idx value = nbr//2 placed at
    [s%16 (replicated across the 8 16-partition groups), t*128 + 64*h + s//16].
    """
    idxs, pars = [], []
    for c in range(C):
        nb = nbr_pad[c * npc : (c + 1) * npc].reshape(n_tiles, P, DEG)
        a = (nb // 2).astype(np.int16).transpose(0, 2, 1)  # [t, k, n]
        a = a.reshape(n_tiles, 2, 8, 8, 16)  # [t, h, k_l, n_hi, n_lo]
        b = a.transpose(4, 0, 1, 2, 3).reshape(16, n_tiles * P)
        idxs.append(np.ascontiguousarray(np.tile(b, (8, 1))))
        p = (nb & 1).astype(np.uint8).transpose(1, 0, 2).reshape(P, n_tiles * DEG)
        pars.append(np.ascontiguousarray(p))
    return idxs, pars


def _run_layer(nc, table_f16, xt_slices, idxs, pars, W, b, trace=False):
    wt = np.ascontiguousarray(W[:P, :]).astype(np.float16)
    wb = (np.ascontiguousarray(W[P:, :]) / np.float32(DEG)).astype(np.float16)
    bia = np.asarray(b, dtype=np.float32).reshape(P, 1)
    t2 = table_f16.reshape(table_f16.shape[0] // 2, 2 * P)
    in_maps = []
    for c in range(C):
        in_maps.append(
            {
                "t2": t2,
                "xt": xt_slices[c],
                "idx": idxs[c],
                "par": pars[c],
                "wt": wt,
                "wb": wb,
                "bia": bia,
            }
        )
    res = run_bass_kernel_spmd(nc, in_maps, core_ids=list(range(C)), trace=trace)
    # h^T stitched: [128, n_full] f32
    ht = np.concatenate([res.results[c]["out"] for c in range(C)], axis=1)
    return ht, res


LAST_RUNS = []


def kernel(x, neighbors, W1, b1, W2, b2):
    N, D = x.shape
    assert D == P
    npc = -(-N // (C * P)) * P  # rows per core, padded to 128
    n_full = C * npc
    n_tiles = npc // P

    xp = np.zeros((n_full, P), dtype=np.float32)
    xp[:N] = np.asarray(x, dtype=np.float32)
    nbr_pad = np.zeros((n_full, DEG), dtype=np.int64)
    nbr_pad[:N] = np.asarray(neighbors)

    idxs, pars = _make_idx_par(nbr_pad, npc, n_tiles)
    nc = build_layer_nc(n_tiles, n_full)

    x16 = xp.astype(np.float16)
    xt1 = [
        np.ascontiguousarray(x16[c * npc : (c + 1) * npc].T) for c in range(C)
    ]
    h1t, r1 = _run_layer(nc, x16, xt1, idxs, pars, W1, b1)

    h1_16 = h1t.astype(np.float16)  # [128, n_full]
    table2 = np.ascontiguousarray(h1_16.T)  # [n_full, 128] node-major
    xt2 = [
        np.ascontiguousarray(h1_16[:, c * npc : (c + 1) * npc]) for c in range(C)
    ]
    h2t, r2 = _run_layer(nc, table2, xt2, idxs, pars, W2, b2)

    LAST_RUNS[:] = [r1, r2]
    return np.ascontiguousarray(h2t.T[:N]).astype(np.float32)
